# revision 1
# baseline (speedup 1.0000x reference)
"""Bidirectional Mamba layer for Trainium2 (8 NeuronCores).

Sharding: core = (batch b in {0,1}) x (direction in {fwd,bwd}) x (d_inner half).
All 8 cores run one SPMD program with per-core input arrays; there are no
cross-core collectives. The host flips the sequence for the backward direction,
permutes u-channels so each core's own d_inner half is always channel-tiles
0..5, pre-tiles every weight matrix so each SBUF destination loads with one
large contiguous DMA (the HWDGE unit costs ~625ns per DMA instruction), and
sums the row-parallel + fwd/bwd partial outputs during the gather.

Per-core program:
  A) in_proj (fp32r matmuls), causal depthwise conv as 4 diagonal-matmul taps
     on the tensor engine (diagonals built on the idle vector engine), SiLU;
     xproj accumulated incrementally as each u-tile is produced;
     softplus(dt_proj + bias) via exp+ln; w = delta*u.
  B) selective scan: for each (d-tile, state n): dA = exp(delta * A[:,n]) on
     the scalar engine, dBu = w * bcast(B_n) on vector, hardware
     tensor_tensor_scan over t, g = h * bcast(C_n), and y += I.T @ g
     accumulated in PSUM by the tensor engine (the sum over n).
  C) y = (y + u*D) * silu(z);  D) out_proj partial, summed on host.
"""
import sys

sys.path.insert(0, "/opt/trn_rl_repo")

from contextlib import ExitStack

import ml_dtypes
import numpy as np

import concourse.bass as bass
import concourse.mybir as mybir
import concourse.tile as tile
from concourse import bacc
from concourse.bass_utils import run_bass_kernel_spmd

D_MODEL = 768
D_STATE = 16
D_INNER = 1536
DT_RANK = 48
D_CONV = 4
BATCH = 2
SEQ = 1024
DH = D_INNER // 2          # 768 scan channels per core
P = 128
KM = D_MODEL // P          # 6 k-tiles over d_model
MU = D_INNER // P          # 12 m-tiles for full u
MH = DH // P               # 6 m-tiles for the half (z, delta, scan, out_proj k)
TH = SEQ // 512            # 2 t-halves for matmul free dim

F32 = mybir.dt.float32
F32R = mybir.dt.float32r
BF16 = mybir.dt.bfloat16
AF = mybir.ActivationFunctionType
OP = mybir.AluOpType

_CACHE = {}


def _build():
    nc = bacc.Bacc("TRN2", target_bir_lowering=False, debug=False)

    xT = nc.dram_tensor("xT", [P, KM, SEQ], F32R, kind="ExternalInput")
    wuX = nc.dram_tensor("wuX", [MU, P, KM * P], F32R, kind="ExternalInput")
    wzX = nc.dram_tensor("wzX", [MH, P, KM * P], F32R, kind="ExternalInput")
    convw = nc.dram_tensor("convw", [P, MU, D_CONV], F32, kind="ExternalInput")
    cbias = nc.dram_tensor("cbias", [P, MU], F32, kind="ExternalInput")
    xpX = nc.dram_tensor("xpX", [P, MU, 80], F32R, kind="ExternalInput")
    dtwT = nc.dram_tensor("dtwT", [DT_RANK + 1, DH], F32R, kind="ExternalInput")
    ones1 = nc.dram_tensor("ones1", [1, SEQ], F32R, kind="ExternalInput")
    Amat = nc.dram_tensor("Amat", [P, MH, D_STATE], F32, kind="ExternalInput")
    Dsk = nc.dram_tensor("Dsk", [P, MH], F32, kind="ExternalInput")
    owX = nc.dram_tensor("owX", [P, MH, KM, P], F32R, kind="ExternalInput")
    eye = nc.dram_tensor("eye", [P, P], F32R, kind="ExternalInput")
    zpad = nc.dram_tensor("zpad", [P, D_CONV - 1], F32R, kind="ExternalInput")
    zb = nc.dram_tensor("zb", [P, 2], BF16, kind="ExternalInput")
    outp = nc.dram_tensor("outp", [D_MODEL, SEQ], F32, kind="ExternalOutput")

    with tile.TileContext(nc) as tc, ExitStack() as top:
        persist = top.enter_context(tc.tile_pool(name="persist", bufs=1))
        ops_pool = top.enter_context(tc.tile_pool(name="ps_o", bufs=2, space="PSUM"))
        dram = top.enter_context(tc.tile_pool(name="dram", bufs=1, space="DRAM"))
        us = [persist.tile([P, SEQ], F32R, tag=f"us{m}", name=f"us{m}")
              for m in range(MH)]
        sz = [persist.tile([P, SEQ], F32, tag=f"sz{m}", name=f"sz{m}")
              for m in range(MH)]
        delta_all = persist.tile([P, MH, SEQ], BF16, tag="dl")
        wdu = [persist.tile([P, SEQ], BF16, tag=f"w{m}", name=f"w{m}")
               for m in range(MH)]
        A_sb = persist.tile([P, MH, D_STATE], F32, tag="A")
        cb_sb = persist.tile([P, MU], F32, tag="cb")
        dsk_sb = persist.tile([P, MH], F32, tag="dsk")
        cw_sb = persist.tile([P, MU, D_CONV], F32, tag="cw")
        eye_sb = persist.tile([P, P], F32R, tag="eye")
        ow_sb = persist.tile([P, MH, KM, P], F32R, tag="ow")
        eye_b = persist.tile([P, P], BF16, tag="eyeb")
        bcd = dram.tile([2 * D_STATE, SEQ], BF16, tag="bc")
        nc.sync.dma_start(out=A_sb, in_=Amat[:, :, :])
        nc.sync.dma_start(out=dsk_sb, in_=Dsk[:, :])
        nc.sync.dma_start(out=cb_sb, in_=cbias[:, :])
        nc.sync.dma_start(out=cw_sb, in_=convw[:, :, :])
        nc.sync.dma_start(out=eye_sb, in_=eye[:, :])

        # ---------------- Phase A: projections ----------------
        with ExitStack() as pa:
            xs_pool = pa.enter_context(tc.tile_pool(name="xs", bufs=1))
            wpool = pa.enter_context(tc.tile_pool(name="wstream", bufs=4))
            djpool = pa.enter_context(tc.tile_pool(name="djp", bufs=8))
            ubuf_pool = pa.enter_context(tc.tile_pool(name="ubuf", bufs=1))
            uoth_pool = pa.enter_context(tc.tile_pool(name="uoth", bufs=2))
            ps_a = pa.enter_context(tc.tile_pool(name="ps_a", bufs=2, space="PSUM"))
            ps_xp = pa.enter_context(tc.tile_pool(name="ps_xp", bufs=1, space="PSUM"))
            misc = pa.enter_context(tc.tile_pool(name="misc_a", bufs=1))

            xs_all = xs_pool.tile([P, KM, SEQ], F32R, tag="xs")
            xs = [xs_all[:, k, :] for k in range(KM)]
            # first x chunk and first weight tile land before the rest so the
            # tensor engine starts early
            nc.sync.dma_start(out=xs_all[:, 0, :], in_=xT[:, 0, :])
            wu0 = wpool.tile([P, KM * P], F32R, tag="w")
            nc.sync.dma_start(out=wu0, in_=wuX[0, :, :])
            for k in range(1, KM):
                nc.sync.dma_start(out=xs_all[:, k, :], in_=xT[:, k, :])

            xp_all = misc.tile([P, MU, 80], F32R, tag="xp")
            nc.sync.dma_start(out=xp_all, in_=xpX[:, :, :])

            # two conv staging buffers; zero pad written once each
            ubufs = [ubuf_pool.tile([P, D_CONV - 1 + SEQ], F32R, tag=f"ubuf{i}",
                                    name=f"ubuf{i}") for i in range(2)]
            for i in range(2):
                nc.sync.dma_start(out=ubufs[i][:, 0:D_CONV - 1], in_=zpad[:, :])

            # xproj accumulators, fed incrementally as each u-tile is made
            psx = [ps_xp.tile([80, 512], F32, tag=f"psx{th}", name=f"psx{th}")
                   for th in range(TH)]

            # u path: in_proj -> causal conv -> silu -> xproj contribution
            for m in range(MU):
                if m == 0:
                    wu_m = wu0
                else:
                    wu_m = wpool.tile([P, KM * P], F32R, tag="w")
                    nc.sync.dma_start(out=wu_m, in_=wuX[m, :, :])
                ub = ubufs[m % 2]
                for th in range(TH):
                    ps = ps_a.tile([P, 512], F32, tag="ps")
                    for k in range(KM):
                        nc.tensor.matmul(ps, wu_m[:, k * P:(k + 1) * P],
                                         xs[k][:, th * 512:(th + 1) * 512],
                                         start=(k == 0), stop=(k == KM - 1))
                    nc.scalar.copy(
                        out=ub[:, D_CONV - 1 + th * 512:D_CONV - 1 + (th + 1) * 512],
                        in_=ps)
                # depthwise causal conv as 4 diagonal-matmul taps;
                # diagonals built on the (idle) vector engine
                ut = us[m] if m < MH else uoth_pool.tile([P, SEQ], F32R,
                                                         tag="uo", name="uo")
                djs = []
                for j in range(D_CONV):
                    dj = djpool.tile([P, P], F32R, tag="dj")
                    nc.vector.tensor_scalar_mul(dj, eye_sb, cw_sb[:, m, j:j + 1])
                    djs.append(dj)
                for th in range(TH):
                    psc = ps_a.tile([P, 512], F32, tag="ps")
                    for j in range(D_CONV):
                        nc.tensor.matmul(psc, djs[j],
                                         ub[:, j + th * 512:j + th * 512 + 512],
                                         start=(j == 0), stop=(j == D_CONV - 1))
                    nc.scalar.activation(out=ut[:, th * 512:(th + 1) * 512], in_=psc,
                                         func=AF.Silu, bias=cb_sb[:, m:m + 1])
                # xproj: accumulate this k=m contribution into psx
                for th in range(TH):
                    nc.tensor.matmul(psx[th], xp_all[:, m, :],
                                     ut[:, th * 512:(th + 1) * 512],
                                     start=(m == 0), stop=(m == MU - 1))

            # z path: in_proj half + silu (PE fills the delta/ACT window)
            for mz in range(MH):
                wz_m = wpool.tile([P, KM * P], F32R, tag="w")
                nc.sync.dma_start(out=wz_m, in_=wzX[mz, :, :])
                for th in range(TH):
                    ps = ops_pool.tile([P, 512], F32, tag="ps")
                    for k in range(KM):
                        nc.tensor.matmul(ps, wz_m[:, k * P:(k + 1) * P],
                                         xs[k][:, th * 512:(th + 1) * 512],
                                         start=(k == 0), stop=(k == KM - 1))
                    nc.scalar.activation(out=sz[mz][:, th * 512:(th + 1) * 512],
                                         in_=ps, func=AF.Silu)

            # x_dbl out of PSUM: fp32 copy (B/C rows) + fp32r copy (dt rows)
            xd_bc = misc.tile([80, SEQ], BF16, tag="xdbc")
            xd_r = misc.tile([DT_RANK + 1, SEQ], F32R, tag="xdr")
            for th in range(TH):
                # non-zero-base partition slices are limited to 32 partitions
                nc.scalar.copy(out=xd_bc[32:64, th * 512:(th + 1) * 512],
                               in_=psx[th][32:64, :])
                nc.scalar.copy(out=xd_bc[64:80, th * 512:(th + 1) * 512],
                               in_=psx[th][64:80, :])
                nc.scalar.copy(out=xd_r[0:DT_RANK, th * 512:(th + 1) * 512],
                               in_=psx[th][0:DT_RANK, :])

            # delta = softplus(dt @ dt_w.T + dt_b) = ln(exp(.) + 1), batched:
            # dt_b rides as an extra contraction row against a ones-row, so
            # exp/ln run as two whole-width ACT ops (no table thrash)
            nc.sync.dma_start(out=xd_r[DT_RANK:DT_RANK + 1, :], in_=ones1[:, :])
            dtw_sb = misc.tile([DT_RANK + 1, DH], F32R, tag="dtw")
            nc.sync.dma_start(out=dtw_sb, in_=dtwT[:, :])
            ps_dt = pa.enter_context(tc.tile_pool(name="ps_dt", bufs=1,
                                                  space="PSUM"))
            for th in range(TH):
                e1 = misc.tile([P, MH, 512], BF16, tag="sp_e", bufs=2)
                for mb in range(MH // 2):
                    psd2 = ps_dt.tile([P, 2, 512], F32, tag="psd")
                    for mi in range(2):
                        m = 2 * mb + mi
                        nc.tensor.matmul(psd2[:, mi, :],
                                         dtw_sb[:, m * P:(m + 1) * P],
                                         xd_r[:, th * 512:(th + 1) * 512],
                                         start=True, stop=True)
                    nc.scalar.activation(out=e1[:, 2 * mb:2 * mb + 2, :],
                                         in_=psd2, func=AF.Exp)
                nc.scalar.activation(
                    out=delta_all[:, :, th * 512:(th + 1) * 512],
                    in_=e1, func=AF.Ln, bias=1.0)

            # w = delta * u  (scan-half channels only)
            for m in range(MH):
                nc.vector.tensor_tensor(out=wdu[m], in0=delta_all[:, m, :],
                                        in1=us[m], op=OP.mult)

            # stage B and C rows to DRAM for partition-broadcast reads
            nc.sync.dma_start(out=bcd[:, :], in_=xd_bc[DT_RANK:80, :])

        nc.sync.dma_start(out=ow_sb, in_=owX[:, :, :, :])
        nc.scalar.copy(out=eye_b, in_=eye_sb)

        late = top.enter_context(tc.tile_pool(name="late", bufs=1))
        yf = [late.tile([P, SEQ], F32R, tag=f"yf{m}", name=f"yf{m}")
              for m in range(MH)]
        o1 = [late.tile([P, SEQ], F32, tag=f"o1{m}", name=f"o1{m}")
              for m in range(KM)]

        # ---------------- Phase B: selective scan ----------------
        _CACHE0 = {}
        with ExitStack() as pb:
            bc_pool = pb.enter_context(tc.tile_pool(name="bc", bufs=2))
            sc_pool = pb.enter_context(tc.tile_pool(name="scan", bufs=2))
            ps_y = pb.enter_context(tc.tile_pool(name="ps_y", bufs=1, space="PSUM"))
            NDSET = 2
            DPS = MH // NDSET  # 3 d-tiles per set
            for ds in range(NDSET):
                yps = [ps_y.tile([P, SEQ], F32, tag=f"y{i}", name=f"y{i}")
                       for i in range(DPS)]
                NG = 2
                for np_ in range(D_STATE // NG):
                    n0 = NG * np_
                    # rows {n0..n0+3} and {16+n0..}: [bc-pair, n-group, t]
                    bcg = bc_pool.tile([P, 2, NG, SEQ], BF16, tag="bc2")
                    srcg = bass.AP(
                        tensor=bcd.tensor, offset=bcd.offset + n0 * SEQ,
                        ap=[[0, P], [D_STATE * SEQ, 2], [SEQ, NG], [1, SEQ]])
                    nc.sync.dma_start(out=bcg, in_=srcg)
                    for i in range(DPS):
                        m = ds * DPS + i
                        # rows padded to SEQ+2 with zero boundary columns so a
                        # single chained scan covers both n's (state resets to
                        # zero through the dA=0, dBu=0 boundary elements);
                        # even row stride keeps bf16 ops 4B-aligned
                        SP2 = SEQ + 2
                        dbu4 = sc_pool.tile([P, NG, SP2], BF16, tag="dbu")
                        da4 = sc_pool.tile([P, NG, SP2], BF16, tag="da")
                        ctr = _CACHE0.setdefault("bz", 0)
                        if ctr < 2:
                            _CACHE0["bz"] = ctr + 1
                            for tzi in (dbu4, da4):
                                nc.sync.dma_start(
                                    out=tzi[:, :, SEQ:SP2],
                                    in_=zb[:, :].unsqueeze(1)
                                        .broadcast_to([P, NG, 2]))
                        nc.vector.tensor_tensor(
                            out=dbu4[:, :, 0:SEQ],
                            in0=wdu[m].unsqueeze(1).broadcast_to([P, NG, SEQ]),
                            in1=bcg[:, 0, :, :], op=OP.mult)
                        for j in range(NG):
                            nc.scalar.activation(out=da4[:, j, 0:SEQ],
                                                 in_=delta_all[:, m, :],
                                                 func=AF.Exp,
                                                 scale=A_sb[:, m, n0 + j:n0 + j + 1])
                        h4 = sc_pool.tile([P, NG, SP2], BF16, tag="h")
                        nc.vector.tensor_tensor_scan(
                            out=h4.rearrange("p a b -> p (a b)"),
                            data0=da4.rearrange("p a b -> p (a b)"),
                            data1=dbu4.rearrange("p a b -> p (a b)"),
                            initial=0.0, op0=OP.mult, op1=OP.add)
                        g4 = sc_pool.tile([P, NG, SEQ], BF16, tag="g")
                        nc.vector.tensor_tensor(out=g4, in0=h4[:, :, 0:SEQ],
                                                in1=bcg[:, 1, :, :], op=OP.mult)
                        for j in range(NG):
                            for th in range(TH):
                                nc.tensor.matmul(
                                    yps[i][:, th * 512:(th + 1) * 512], eye_b,
                                    g4[:, j, th * 512:(th + 1) * 512],
                                    start=(n0 + j == 0), stop=False)
                # Phase C for this d-set: y += u*D on PE, then gate with silu(z)
                for i in range(DPS):
                    m = ds * DPS + i
                    dD = sc_pool.tile([P, P], F32R, tag="dD", bufs=3)
                    nc.vector.tensor_scalar_mul(dD, eye_sb, dsk_sb[:, m:m + 1])
                    for th in range(TH):
                        nc.tensor.matmul(yps[i][:, th * 512:(th + 1) * 512], dD,
                                         us[m][:, th * 512:(th + 1) * 512],
                                         start=False, stop=True)
                    nc.vector.tensor_tensor(out=yf[m], in0=yps[i], in1=sz[m],
                                            op=OP.mult)
                if ds == 0:
                    for mo in range(KM):
                        for th in range(TH):
                            psg = ops_pool.tile([P, 512], F32, tag="ps")
                            for k in range(DPS):
                                nc.tensor.matmul(
                                    psg, ow_sb[:, k, mo, :],
                                    yf[k][:, th * 512:(th + 1) * 512],
                                    start=(k == 0), stop=(k == DPS - 1))
                            nc.scalar.copy(
                                out=o1[mo][:, th * 512:(th + 1) * 512], in_=psg)

        # ---------------- Phase D: out_proj ----------------
        with ExitStack() as pd:
            ost = pd.enter_context(tc.tile_pool(name="ost", bufs=2))
            for m in range(KM):
                ot = ost.tile([P, SEQ], F32, tag="ot")
                for th in range(TH):
                    ps = ops_pool.tile([P, 512], F32, tag="ps")
                    for k in range(DPS, MH):
                        nc.tensor.matmul(ps, ow_sb[:, k, m, :],
                                         yf[k][:, th * 512:(th + 1) * 512],
                                         start=(k == DPS), stop=(k == MH - 1))
                    nc.vector.tensor_tensor(
                        out=ot[:, th * 512:(th + 1) * 512], in0=ps,
                        in1=o1[m][:, th * 512:(th + 1) * 512], op=OP.add)
                nc.sync.dma_start(out=outp[m * P:(m + 1) * P, :], in_=ot)

    nc.finalize()
    return nc


def _prep_core(x, prm, b, direction, half):
    """Build the per-core input map. prm maps param name -> array."""
    xb = np.ascontiguousarray(x[b])                # (L, D_MODEL)
    if direction == 1:
        xb = np.ascontiguousarray(xb[::-1])
    in_w = prm["in_w"]
    conv_w = prm["conv_w"]
    conv_b = prm["conv_b"]
    xproj_w = prm["xproj_w"]
    dt_w = prm["dt_w"]
    dt_b = prm["dt_b"]
    Alog = prm["Alog"]
    Dp = prm["D"]
    out_w = prm["out_w"]

    own = np.arange(half * DH, (half + 1) * DH)
    oth = np.arange((1 - half) * DH, (2 - half) * DH)
    perm = np.concatenate([own, oth])              # u-channel permutation

    wu = in_w[0:D_INNER][perm]                     # (1536, 768), own half first
    wz = in_w[D_INNER:2 * D_INNER][own]            # (768, 768)
    cw = conv_w[perm]                              # (1536, 4)
    A = -np.exp(Alog[own])                         # (768, 16)

    def lhs_tiles(mat_t, kk, mm):
        # (K*P, M*P) -> (mm, P, kk*P): per m-tile, partition-contiguous rows
        return np.ascontiguousarray(
            mat_t.reshape(kk, P, mm, P).transpose(2, 1, 0, 3).reshape(mm, P, kk * P))

    return {
        "xT": np.ascontiguousarray(xb.T.reshape(KM, P, SEQ).transpose(1, 0, 2)),
        "wuX": lhs_tiles(wu.T, KM, MU),
        "wzX": lhs_tiles(wz.T, KM, MH),
        "convw": np.ascontiguousarray(cw.reshape(MU, P, D_CONV).transpose(1, 0, 2)),
        "cbias": np.ascontiguousarray(conv_b[perm].reshape(MU, P).T),
        "xpX": np.ascontiguousarray(
            xproj_w[:, perm].T.reshape(MU, P, 80).transpose(1, 0, 2)),
        "dtwT": np.ascontiguousarray(
            np.vstack([dt_w[own].T, dt_b[own][None, :]])),
        "ones1": np.ones((1, SEQ), dtype=np.float32),
        "Amat": np.ascontiguousarray(A.reshape(MH, P, D_STATE).transpose(1, 0, 2)),
        "Dsk": np.ascontiguousarray(Dp[own].reshape(MH, P).T),
        "owX": np.ascontiguousarray(
            out_w[:, own].T.reshape(MH, P, KM, P).transpose(1, 0, 2, 3)),
        "eye": np.eye(P, dtype=np.float32),
        "zpad": np.zeros((P, D_CONV - 1), dtype=np.float32),
        "zb": np.zeros((P, 2), dtype=ml_dtypes.bfloat16),
    }


def _in_maps(inputs):
    x = inputs["x"]
    maps = []
    for b in range(BATCH):
        for direction in range(2):
            pfx = "f" if direction == 0 else "b"
            prm = {k: inputs[f"{pfx}_{k}"] for k in
                   ("in_w", "conv_w", "conv_b", "xproj_w", "dt_w", "dt_b",
                    "Alog", "D", "out_w")}
            for half in range(2):
                maps.append(_prep_core(x, prm, b, direction, half))
    return maps


def kernel(**inputs):
    inputs = {k: np.asarray(v, dtype=np.float32) for k, v in inputs.items()}
    nc = _CACHE.get("nc")
    if nc is None:
        nc = _build()
        _CACHE["nc"] = nc
    maps = _in_maps(inputs)
    res = run_bass_kernel_spmd(nc, maps, list(range(8)),
                               **_CACHE.get("run_kwargs", {}))
    _CACHE["last_results"] = res
    out = np.zeros((BATCH, SEQ, D_MODEL), dtype=np.float32)
    ci = 0
    for b in range(BATCH):
        for direction in range(2):
            for half in range(2):
                part = res.results[ci]["outp"].T          # (SEQ, D_MODEL)
                if direction == 1:
                    part = part[::-1]
                out[b] += part
                ci += 1
    return out



# revision 24
# speedup vs baseline: 1.1637x; 1.1637x over previous
"""Bidirectional Mamba layer for Trainium2 (8 NeuronCores).

Sharding: core = (batch b in {0,1}) x (direction in {fwd,bwd}) x (d_inner half).
All 8 cores run one SPMD program with per-core input arrays; no cross-core
collectives. The host flips the sequence for the backward direction, permutes
u-channels so each core's own d_inner half is channel-tiles 0..5, pre-tiles
weights for single-DMA loads, and sums partial outputs during the gather.

Per-core program (engine assignment tuned against the TimelineSim cost model):
  A) own-half in_proj as fp32r matmuls -> causal depthwise conv as a
     scalar_tensor_tensor MAC chain on the vector engine -> SiLU;
     other-half in_proj + conv (which only feed x_proj) in fp8 DoubleRow
     matmuls (weights pre-scaled x64 on host, un-scaled in the SiLU);
     xproj accumulated incrementally as each u-tile is produced.
  T) delta = Softplus(dt_proj) directly (dt_b rides as an extra contraction
     row); w = delta*u on the pool engine; B/C rows staged to DRAM in two
     chunks so the first scan group's broadcast load starts early.
  B) selective scan per (d-tile, 2-state group): dA = exp(delta * A[:,n]) on
     the scalar engine, dBu = w * bcast(B_n) on vector, hardware
     tensor_tensor_scan over t on vector, g = h * bcast(C_n) on vector OR the
     pool engine (static split tuned to balance both), y += I.T @ g in PSUM
     on the tensor engine. z-projection matmuls run on the otherwise idle
     tensor engine inside this phase.
  C) y += u*D via diag-matmul (diags built on pool), yf = y * silu(z);
  D) out_proj in two halves; partials summed on host.
"""
import sys

sys.path.insert(0, "/opt/trn_rl_repo")

from contextlib import ExitStack

import ml_dtypes
import numpy as np

import concourse.bass as bass
import concourse.mybir as mybir
import concourse.tile as tile
from concourse import bacc
from concourse.bass_utils import run_bass_kernel_spmd

D_MODEL = 768
D_STATE = 16
D_INNER = 1536
DT_RANK = 48
D_CONV = 4
BATCH = 2
SEQ = 1024
DH = D_INNER // 2          # 768 scan channels per core
P = 128
KM = D_MODEL // P          # 6 k-tiles over d_model
KP = KM // 2               # 3 fp8 DoubleRow k-pairs
MU = D_INNER // P          # 12 m-tiles for full u
MH = DH // P               # 6 m-tiles for the half
TH = SEQ // 512            # 2 t-halves for matmul free dim

F32 = mybir.dt.float32
F32R = mybir.dt.float32r
BF16 = mybir.dt.bfloat16
FP8 = mybir.dt.float8e4
AF = mybir.ActivationFunctionType
OP = mybir.AluOpType
PM = mybir.MatmulPerfMode

FP8_WSCALE = 64.0          # host pre-scale on fp8 weights / conv diags

NG = 2                     # states chained per scan op
NPG = D_STATE // NG        # 8 n-groups
G_POOL_PER_DS = 19         # of the 24 (np_, i) units per d-set, how many
                           # g-mults go to the pool engine (first ones)

_CACHE = {}


def _build():
    nc = bacc.Bacc("TRN2", target_bir_lowering=False, debug=False)

    xT = nc.dram_tensor("xT", [P, KM, SEQ], F32R, kind="ExternalInput")
    xT8 = nc.dram_tensor("xT8", [P, KM, SEQ], FP8, kind="ExternalInput")
    wuX = nc.dram_tensor("wuX", [MH, P, KM * P], F32R, kind="ExternalInput")
    wuX8 = nc.dram_tensor("wuX8", [MH, P, KP, 2, P], FP8, kind="ExternalInput")
    wzX = nc.dram_tensor("wzX", [MH, P, KM * P], F32R, kind="ExternalInput")
    convw = nc.dram_tensor("convw", [P, MH, D_CONV], F32, kind="ExternalInput")
    convw8 = nc.dram_tensor("convw8", [MH, P, 2, 2, P], FP8, kind="ExternalInput")
    cbias = nc.dram_tensor("cbias", [P, MU], F32, kind="ExternalInput")
    xpX = nc.dram_tensor("xpX", [P, MU, 80], F32R, kind="ExternalInput")
    dtwT = nc.dram_tensor("dtwT", [DT_RANK + 1, DH], F32R, kind="ExternalInput")
    ones1 = nc.dram_tensor("ones1", [1, SEQ], F32R, kind="ExternalInput")
    Amat = nc.dram_tensor("Amat", [P, MH, D_STATE], F32, kind="ExternalInput")
    Dsk = nc.dram_tensor("Dsk", [P, MH], F32, kind="ExternalInput")
    owX = nc.dram_tensor("owX", [P, MH, KM, P], BF16, kind="ExternalInput")
    eye = nc.dram_tensor("eye", [P, P], F32R, kind="ExternalInput")
    zpad = nc.dram_tensor("zpad", [P, D_CONV - 1], F32R, kind="ExternalInput")
    zpad8 = nc.dram_tensor("zpad8", [P, D_CONV - 1], FP8, kind="ExternalInput")
    zb = nc.dram_tensor("zb", [P, 2], BF16, kind="ExternalInput")
    outp = nc.dram_tensor("outp", [D_MODEL, SEQ], F32, kind="ExternalOutput")

    with tile.TileContext(nc) as tc, ExitStack() as top:
        persist = top.enter_context(tc.tile_pool(name="persist", bufs=1))
        ops_pool = top.enter_context(tc.tile_pool(name="ps_o", bufs=2, space="PSUM"))
        dram = top.enter_context(tc.tile_pool(name="dram", bufs=1, space="DRAM"))
        us = [persist.tile([P, SEQ], F32R, tag=f"us{m}", name=f"us{m}")
              for m in range(MH)]
        zraw = persist.tile([P, MH, SEQ], BF16, tag="zr")
        sz = [persist.tile([P, SEQ], BF16, tag=f"sz{m}", name=f"sz{m}")
              for m in range(MH)]
        delta_all = persist.tile([P, MH, SEQ], BF16, tag="dl")
        wdu = [persist.tile([P, SEQ], BF16, tag=f"w{m}", name=f"w{m}")
               for m in range(MH)]
        A_sb = persist.tile([P, MH, D_STATE], F32, tag="A")
        cb_sb = persist.tile([P, MU], F32, tag="cb")
        dsk_sb = persist.tile([P, MH], F32, tag="dsk")
        cw_sb = persist.tile([P, MH, D_CONV], F32, tag="cw")
        eye_sb = persist.tile([P, P], F32R, tag="eye")
        ow_sb = persist.tile([P, MH, KM, P], BF16, tag="ow")
        eye_b = persist.tile([P, P], BF16, tag="eyeb")
        # x stays resident: the z-projection reads it inside phase B
        xs_all = persist.tile([P, KM, SEQ], F32R, tag="xs")
        bcd = dram.tile([2 * D_STATE, SEQ], BF16, tag="bc")
        nc.sync.dma_start(out=A_sb, in_=Amat[:, :, :])
        nc.sync.dma_start(out=dsk_sb, in_=Dsk[:, :])
        nc.sync.dma_start(out=cb_sb, in_=cbias[:, :])
        nc.sync.dma_start(out=cw_sb, in_=convw[:, :, :])
        nc.sync.dma_start(out=eye_sb, in_=eye[:, :])

        # ---------------- Phase A: projections ----------------
        with ExitStack() as pa:
            x8_pool = pa.enter_context(tc.tile_pool(name="xs8", bufs=1))
            wpool = pa.enter_context(tc.tile_pool(name="wstream", bufs=4))
            w8pool = pa.enter_context(tc.tile_pool(name="w8s", bufs=2))
            c8pool = pa.enter_context(tc.tile_pool(name="c8s", bufs=2))
            accpool = pa.enter_context(tc.tile_pool(name="acc", bufs=4))
            ubuf_pool = pa.enter_context(tc.tile_pool(name="ubuf", bufs=1))
            uoth_pool = pa.enter_context(tc.tile_pool(name="uoth", bufs=2))
            ps_a = pa.enter_context(tc.tile_pool(name="ps_a", bufs=2, space="PSUM"))
            ps_xp = pa.enter_context(tc.tile_pool(name="ps_xp", bufs=1, space="PSUM"))
            misc = pa.enter_context(tc.tile_pool(name="misc_a", bufs=1))

            xs = [xs_all[:, k, :] for k in range(KM)]
            x8_all = x8_pool.tile([P, KM, SEQ], FP8, tag="xs8")
            # first x chunk and first weight tile land before the rest so the
            # tensor engine starts early
            nc.sync.dma_start(out=xs_all[:, 0, :], in_=xT[:, 0, :])
            wu0 = wpool.tile([P, KM * P], F32R, tag="w")
            nc.sync.dma_start(out=wu0, in_=wuX[0, :, :])
            for k in range(1, KM):
                nc.sync.dma_start(out=xs_all[:, k, :], in_=xT[:, k, :])
            nc.sync.dma_start(out=x8_all, in_=xT8[:, :, :])

            xp_all = misc.tile([P, MU, 80], F32R, tag="xp")
            nc.sync.dma_start(out=xp_all, in_=xpX[:, :, :])

            # conv staging: fp32r for own half, fp8 for other half
            ubufs = [ubuf_pool.tile([P, D_CONV - 1 + SEQ], F32R, tag=f"ubuf{i}",
                                    name=f"ubuf{i}") for i in range(2)]
            ub8s = [ubuf_pool.tile([P, D_CONV - 1 + SEQ], FP8, tag=f"ub8{i}",
                                   name=f"ub8{i}") for i in range(2)]
            for i in range(2):
                nc.sync.dma_start(out=ubufs[i][:, 0:D_CONV - 1], in_=zpad[:, :])
                nc.sync.dma_start(out=ub8s[i][:, 0:D_CONV - 1], in_=zpad8[:, :])

            # xproj accumulators, fed incrementally as each u-tile is made
            psx = [ps_xp.tile([80, 512], F32, tag=f"psx{th}", name=f"psx{th}")
                   for th in range(TH)]

            # u path, own and other halves interleaved per m-tile so the
            # own-half DVE conv chain overlaps the other-half Act chain
            for mi in range(MH):
                # own half: fp32r in_proj -> DVE conv MAC chain -> silu
                m = mi
                if m == 0:
                    wu_m = wu0
                else:
                    wu_m = wpool.tile([P, KM * P], F32R, tag="w")
                    nc.sync.dma_start(out=wu_m, in_=wuX[m, :, :])
                ub = ubufs[m % 2]
                for th in range(TH):
                    ps = ps_a.tile([P, 512], F32, tag="ps")
                    for k in range(KM):
                        nc.tensor.matmul(ps, wu_m[:, k * P:(k + 1) * P],
                                         xs[k][:, th * 512:(th + 1) * 512],
                                         start=(k == 0), stop=(k == KM - 1))
                    nc.scalar.copy(
                        out=ub[:, D_CONV - 1 + th * 512:D_CONV - 1 + (th + 1) * 512],
                        in_=ps)
                # causal depthwise conv: 4-tap MAC chain on the vector
                # engine, ping-ponging between two accumulator slots
                for th in range(TH):
                    base = th * 512
                    acc = accpool.tile([P, 2, 512], F32R, tag="acc")
                    nc.vector.tensor_scalar_mul(acc[:, 0, :], ub[:, base:base + 512],
                                                cw_sb[:, m, 0:1])
                    for j in range(1, D_CONV):
                        nc.vector.scalar_tensor_tensor(
                            out=acc[:, j % 2, :],
                            in0=ub[:, base + j:base + j + 512],
                            scalar=cw_sb[:, m, j:j + 1],
                            in1=acc[:, (j - 1) % 2, :], op0=OP.mult, op1=OP.add)
                    nc.scalar.activation(out=us[m][:, base:base + 512],
                                         in_=acc[:, (D_CONV - 1) % 2, :],
                                         func=AF.Silu, bias=cb_sb[:, m:m + 1])
                for th in range(TH):
                    nc.tensor.matmul(psx[th], xp_all[:, m, :],
                                     us[m][:, th * 512:(th + 1) * 512],
                                     start=(m == 0), stop=False)

                # other half (feeds xproj only): fp8 DoubleRow in_proj+conv
                m = MH + mi
                wu8_m = w8pool.tile([P, KP, 2, P], FP8, tag="w8")
                nc.sync.dma_start(out=wu8_m, in_=wuX8[mi, :, :, :, :])
                c8_m = c8pool.tile([P, 2, 2, P], FP8, tag="c8")
                nc.sync.dma_start(out=c8_m, in_=convw8[mi, :, :, :, :])
                ub8 = ub8s[mi % 2]
                for th in range(TH):
                    ps = ps_a.tile([P, 512], F32, tag="ps")
                    for kp in range(KP):
                        nc.tensor.matmul(
                            ps, wu8_m[:, kp, :, :],
                            x8_all[:, 2 * kp:2 * kp + 2, th * 512:(th + 1) * 512],
                            start=(kp == 0), stop=(kp == KP - 1),
                            perf_mode=PM.DoubleRow)
                    nc.scalar.copy(
                        out=ub8[:, D_CONV - 1 + th * 512:D_CONV - 1 + (th + 1) * 512],
                        in_=ps)
                ut = uoth_pool.tile([P, SEQ], F32R, tag="uo", name="uo")
                for th in range(TH):
                    psc = ps_a.tile([P, 512], F32, tag="ps")
                    for jp in range(2):
                        # taps (2jp, 2jp+1) as one DoubleRow pair; the rhs AP
                        # reads two overlapping shifted windows of ub8
                        src = bass.AP(
                            tensor=ub8.tensor,
                            offset=ub8.offset + 2 * jp + th * 512,
                            ap=[list(ub8.ap[0]), [1, 2], [1, 512]])
                        nc.tensor.matmul(psc, c8_m[:, jp, :, :], src,
                                         start=(jp == 0), stop=(jp == 1),
                                         perf_mode=PM.DoubleRow)
                    nc.scalar.activation(out=ut[:, th * 512:(th + 1) * 512],
                                         in_=psc, func=AF.Silu,
                                         bias=cb_sb[:, m:m + 1],
                                         scale=1.0 / (FP8_WSCALE * FP8_WSCALE))
                for th in range(TH):
                    nc.tensor.matmul(psx[th], xp_all[:, m, :],
                                     ut[:, th * 512:(th + 1) * 512],
                                     start=False, stop=(m == MU - 1))

            # ---------------- Transition: x_dbl, delta, w ----------------
            # x_dbl out of PSUM on the vector engine (Act is busy elsewhere);
            # non-zero-base partition slices are limited to 32 partitions
            xd_bc = misc.tile([80, SEQ], BF16, tag="xdbc")
            xd_r = misc.tile([DT_RANK + 1, SEQ], F32R, tag="xdr")
            for th in range(TH):
                nc.vector.tensor_scalar_add(xd_bc[32:64, th * 512:(th + 1) * 512],
                                            psx[th][32:64, :], 0.0)
                nc.vector.tensor_scalar_add(xd_bc[64:80, th * 512:(th + 1) * 512],
                                            psx[th][64:80, :], 0.0)
                nc.vector.tensor_scalar_add(xd_r[0:DT_RANK, th * 512:(th + 1) * 512],
                                            psx[th][0:DT_RANK, :], 0.0)

            # stage B and C rows to DRAM for partition-broadcast reads; the
            # first scan group's rows go first so its bcg load starts early
            nc.sync.dma_start(out=bcd[0:NG, :], in_=xd_bc[DT_RANK:DT_RANK + NG, :])
            nc.sync.dma_start(out=bcd[D_STATE:D_STATE + NG, :],
                              in_=xd_bc[DT_RANK + D_STATE:DT_RANK + D_STATE + NG, :])
            nc.sync.dma_start(out=bcd[NG:D_STATE, :],
                              in_=xd_bc[DT_RANK + NG:DT_RANK + D_STATE, :])
            nc.sync.dma_start(out=bcd[D_STATE + NG:2 * D_STATE, :],
                              in_=xd_bc[DT_RANK + D_STATE + NG:80, :])

            # delta = softplus(dt @ dt_w.T + dt_b); dt_b rides as an extra
            # contraction row against a ones-row
            nc.sync.dma_start(out=xd_r[DT_RANK:DT_RANK + 1, :], in_=ones1[:, :])
            dtw_sb = misc.tile([DT_RANK + 1, DH], F32R, tag="dtw")
            nc.sync.dma_start(out=dtw_sb, in_=dtwT[:, :])
            ps_dt = pa.enter_context(tc.tile_pool(name="ps_dt", bufs=1,
                                                  space="PSUM"))
            # softplus(x) = ln(exp(x) + 1): all exps batched, then two
            # whole-width Ln ops — exp runs stay contiguous so the activation
            # table isn't thrashed (exp and ln live in different tables)
            e1s = [misc.tile([P, MH, 512], BF16, tag=f"sp_e{th}",
                             name=f"sp_e{th}") for th in range(TH)]
            for th in range(TH):
                for mb in range(MH // 2):
                    psd2 = ps_dt.tile([P, 2, 512], F32, tag="psd")
                    for mi in range(2):
                        m = 2 * mb + mi
                        nc.tensor.matmul(psd2[:, mi, :],
                                         dtw_sb[:, m * P:(m + 1) * P],
                                         xd_r[:, th * 512:(th + 1) * 512],
                                         start=True, stop=True)
                    nc.scalar.activation(out=e1s[th][:, 2 * mb:2 * mb + 2, :],
                                         in_=psd2, func=AF.Exp)
            for th in range(TH):
                nc.scalar.activation(
                    out=delta_all[:, :, th * 512:(th + 1) * 512],
                    in_=e1s[th], func=AF.Ln, bias=1.0)
            # w = delta * u on the vector engine (idle in this window)
            for m in range(MH):
                nc.vector.tensor_tensor(out=wdu[m], in0=delta_all[:, m, :],
                                        in1=us[m], op=OP.mult)

        nc.sync.dma_start(out=ow_sb, in_=owX[:, :, :, :])
        nc.scalar.copy(out=eye_b, in_=eye_sb)

        late = top.enter_context(tc.tile_pool(name="late", bufs=1))
        yf = [late.tile([P, SEQ], BF16, tag=f"yf{m}", name=f"yf{m}")
              for m in range(MH)]
        o1 = [late.tile([P, SEQ], BF16, tag=f"o1{m}", name=f"o1{m}")
              for m in range(KM)]

        # ---------------- Phase B: selective scan ----------------
        _CACHE0 = {}
        with ExitStack() as pb:
            bc_pool = pb.enter_context(tc.tile_pool(name="bc", bufs=3))
            sc_pool = pb.enter_context(tc.tile_pool(name="scan", bufs=2))
            wzstr = pb.enter_context(tc.tile_pool(name="wzstr", bufs=2))
            ps_y = pb.enter_context(tc.tile_pool(name="ps_y", bufs=1, space="PSUM"))
            NDSET = 2
            DPS = MH // NDSET  # 3 d-tiles per set
            SP2 = SEQ + 2
            # z matmul schedule: z-tile zi runs on the PE after (ds=0, np_)
            z_sched = {1: [0, 1], 2: [2, 3], 3: [4], 4: [5]}
            for ds in range(NDSET):
                yps = [ps_y.tile([P, SEQ], F32, tag=f"y{i}", name=f"y{i}")
                       for i in range(DPS)]
                for np_ in range(NPG):
                    n0 = NG * np_
                    # rows {n0..} and {16+n0..}: [bc-pair, n-group, t]
                    bcg = bc_pool.tile([P, 2, NG, SEQ], BF16, tag="bc2")
                    srcg = bass.AP(
                        tensor=bcd.tensor, offset=bcd.offset + n0 * SEQ,
                        ap=[[0, P], [D_STATE * SEQ, 2], [SEQ, NG], [1, SEQ]])
                    nc.sync.dma_start(out=bcg, in_=srcg)
                    for i in range(DPS):
                        m = ds * DPS + i
                        u_idx = np_ * DPS + i
                        # rows padded to SEQ+2 with zero boundary columns so a
                        # single chained scan covers both n's (state resets to
                        # zero through the dA=0, dBu=0 boundary elements)
                        dbu4 = sc_pool.tile([P, NG, SP2], BF16, tag="dbu")
                        da4 = sc_pool.tile([P, NG, SP2], BF16, tag="da")
                        ctr = _CACHE0.setdefault("bz", 0)
                        if ctr < 2:
                            _CACHE0["bz"] = ctr + 1
                            for tzi in (dbu4, da4):
                                nc.sync.dma_start(
                                    out=tzi[:, :, SEQ:SP2],
                                    in_=zb[:, :].unsqueeze(1)
                                        .broadcast_to([P, NG, 2]))
                        nc.vector.tensor_tensor(
                            out=dbu4[:, :, 0:SEQ],
                            in0=wdu[m].unsqueeze(1).broadcast_to([P, NG, SEQ]),
                            in1=bcg[:, 0, :, :], op=OP.mult)
                        for j in range(NG):
                            nc.scalar.activation(out=da4[:, j, 0:SEQ],
                                                 in_=delta_all[:, m, :],
                                                 func=AF.Exp,
                                                 scale=A_sb[:, m, n0 + j:n0 + j + 1])
                        h4 = sc_pool.tile([P, NG, SP2], BF16, tag="h", bufs=3)
                        nc.vector.tensor_tensor_scan(
                            out=h4.rearrange("p a b -> p (a b)"),
                            data0=da4.rearrange("p a b -> p (a b)"),
                            data1=dbu4.rearrange("p a b -> p (a b)"),
                            initial=0.0, op0=OP.mult, op1=OP.add)
                        g4 = sc_pool.tile([P, NG, SEQ], BF16, tag="g", bufs=3)
                        geng = (nc.gpsimd if u_idx < G_POOL_PER_DS
                                else nc.vector)
                        geng.tensor_tensor(out=g4, in0=h4[:, :, 0:SEQ],
                                           in1=bcg[:, 1, :, :], op=OP.mult)
                        for j in range(NG):
                            for th in range(TH):
                                nc.tensor.matmul(
                                    yps[i][:, th * 512:(th + 1) * 512], eye_b,
                                    g4[:, j, th * 512:(th + 1) * 512],
                                    start=(n0 + j == 0), stop=False)
                    # z-projection fills the tensor engine's idle slots in
                    # early phase B (pinned so it doesn't crowd phase A)
                    if ds == 0:
                        for zi in z_sched.get(np_, []):
                            wz_m = wzstr.tile([P, KM * P], F32R, tag="wz")
                            nc.sync.dma_start(out=wz_m, in_=wzX[zi, :, :])
                            with tc.tile_wait_until(0.085 + 0.004 * zi):
                                for th in range(TH):
                                    psz = ops_pool.tile([P, 512], F32, tag="ps")
                                    for k in range(KM):
                                        nc.tensor.matmul(
                                            psz, wz_m[:, k * P:(k + 1) * P],
                                            xs_all[:, k, th * 512:(th + 1) * 512],
                                            start=(k == 0), stop=(k == KM - 1))
                                    nc.scalar.copy(
                                        out=zraw[:, zi, th * 512:(th + 1) * 512],
                                        in_=psz)
                # Phase C for this d-set: y += u*D on PE, silu(z), gate.
                # All six sz silus run in one pinned batch mid phase B so the
                # activation table only swaps silu<->exp twice.
                if ds == 0:
                    with tc.tile_wait_until(0.145):
                        for m in range(MH):
                            nc.scalar.activation(out=sz[m], in_=zraw[:, m, :],
                                                 func=AF.Silu)
                for i in range(DPS):
                    m = ds * DPS + i
                    dD = sc_pool.tile([P, P], F32R, tag="dD", bufs=3)
                    nc.gpsimd.tensor_scalar_mul(dD, eye_sb, dsk_sb[:, m:m + 1])
                    for th in range(TH):
                        nc.tensor.matmul(yps[i][:, th * 512:(th + 1) * 512], dD,
                                         us[m][:, th * 512:(th + 1) * 512],
                                         start=False, stop=True)
                    nc.vector.tensor_tensor(out=yf[m], in0=yps[i], in1=sz[m],
                                            op=OP.mult)
                if ds == 0:
                    for mo in range(KM):
                        for th in range(TH):
                            psg = ops_pool.tile([P, 512], F32, tag="ps")
                            for k in range(DPS):
                                nc.tensor.matmul(
                                    psg, ow_sb[:, k, mo, :],
                                    yf[k][:, th * 512:(th + 1) * 512],
                                    start=(k == 0), stop=(k == DPS - 1))
                            nc.scalar.copy(
                                out=o1[mo][:, th * 512:(th + 1) * 512], in_=psg)

        # ---------------- Phase D: out_proj ----------------
        # second-half contraction + the first-half partial o1 folded in with
        # an extra identity matmul, so no vector-engine adds are needed
        with ExitStack() as pd:
            ost = pd.enter_context(tc.tile_pool(name="ost", bufs=2))
            DPS = MH // 2
            for m in range(KM):
                ot = ost.tile([P, SEQ], F32, tag="ot")
                for th in range(TH):
                    ps = ops_pool.tile([P, 512], F32, tag="ps")
                    for k in range(DPS, MH):
                        nc.tensor.matmul(ps, ow_sb[:, k, m, :],
                                         yf[k][:, th * 512:(th + 1) * 512],
                                         start=(k == DPS), stop=False)
                    nc.tensor.matmul(ps, eye_b,
                                     o1[m][:, th * 512:(th + 1) * 512],
                                     start=False, stop=True)
                    nc.scalar.copy(out=ot[:, th * 512:(th + 1) * 512], in_=ps)
                nc.sync.dma_start(out=outp[m * P:(m + 1) * P, :], in_=ot)

    nc.finalize()
    return nc


def _prep_core(x, prm, b, direction, half):
    """Build the per-core input map. prm maps param name -> array."""
    xb = np.ascontiguousarray(x[b])                # (L, D_MODEL)
    if direction == 1:
        xb = np.ascontiguousarray(xb[::-1])
    in_w = prm["in_w"]
    conv_w = prm["conv_w"]
    conv_b = prm["conv_b"]
    xproj_w = prm["xproj_w"]
    dt_w = prm["dt_w"]
    dt_b = prm["dt_b"]
    Alog = prm["Alog"]
    Dp = prm["D"]
    out_w = prm["out_w"]

    own = np.arange(half * DH, (half + 1) * DH)
    oth = np.arange((1 - half) * DH, (2 - half) * DH)
    perm = np.concatenate([own, oth])              # u-channel permutation

    wu_own = in_w[0:D_INNER][own]                  # (768, 768)
    wu_oth = in_w[0:D_INNER][oth]                  # (768, 768)
    wz = in_w[D_INNER:2 * D_INNER][own]            # (768, 768)
    cw = conv_w[perm]                              # (1536, 4)
    A = -np.exp(Alog[own])                         # (768, 16)

    def lhs_tiles(mat_t, kk, mm):
        # (K*P, M*P) -> (mm, P, kk*P): per m-tile, partition-contiguous rows
        return np.ascontiguousarray(
            mat_t.reshape(kk, P, mm, P).transpose(2, 1, 0, 3).reshape(mm, P, kk * P))

    f8 = ml_dtypes.float8_e4m3fn
    xbT = xb.T                                     # (768, L)
    # other-half in_proj weights, x64, as [m, P(k), kp, pair, P(m)] fp8
    wuo = (wu_oth.T * FP8_WSCALE).reshape(KP, 2, P, MH, P)
    wuX8 = np.ascontiguousarray(wuo.transpose(3, 2, 0, 1, 4)).astype(f8)
    # other-half conv tap-pair diagonals, x64: [m, P, jp, pair, P]
    cw8 = np.zeros((MH, P, 2, 2, P), np.float32)
    cwo = conv_w[oth] * FP8_WSCALE                 # (768, 4)
    for mi in range(MH):
        for jp in range(2):
            for pr in range(2):
                d = np.arange(P)
                cw8[mi, d, jp, pr, d] = cwo[mi * P + d, 2 * jp + pr]

    return {
        "xT": np.ascontiguousarray(xbT.reshape(KM, P, SEQ).transpose(1, 0, 2)),
        "xT8": np.ascontiguousarray(
            xbT.reshape(KM, P, SEQ).transpose(1, 0, 2)).astype(f8),
        "wuX": lhs_tiles(wu_own.T, KM, MH),
        "wuX8": wuX8,
        "wzX": lhs_tiles(wz.T, KM, MH),
        "convw": np.ascontiguousarray(
            cw[:DH].reshape(MH, P, D_CONV).transpose(1, 0, 2)),
        "convw8": cw8.astype(f8),
        "cbias": np.ascontiguousarray(conv_b[perm].reshape(MU, P).T),
        "xpX": np.ascontiguousarray(
            xproj_w[:, perm].T.reshape(MU, P, 80).transpose(1, 0, 2)),
        "dtwT": np.ascontiguousarray(
            np.vstack([dt_w[own].T, dt_b[own][None, :]])),
        "ones1": np.ones((1, SEQ), dtype=np.float32),
        "Amat": np.ascontiguousarray(A.reshape(MH, P, D_STATE).transpose(1, 0, 2)),
        "Dsk": np.ascontiguousarray(Dp[own].reshape(MH, P).T),
        "owX": np.ascontiguousarray(
            out_w[:, own].T.reshape(MH, P, KM, P).transpose(1, 0, 2, 3))
        .astype(ml_dtypes.bfloat16),
        "eye": np.eye(P, dtype=np.float32),
        "zpad": np.zeros((P, D_CONV - 1), dtype=np.float32),
        "zpad8": np.zeros((P, D_CONV - 1), dtype=f8),
        "zb": np.zeros((P, 2), dtype=ml_dtypes.bfloat16),
    }


def _in_maps(inputs):
    x = inputs["x"]
    maps = []
    for b in range(BATCH):
        for direction in range(2):
            pfx = "f" if direction == 0 else "b"
            prm = {k: inputs[f"{pfx}_{k}"] for k in
                   ("in_w", "conv_w", "conv_b", "xproj_w", "dt_w", "dt_b",
                    "Alog", "D", "out_w")}
            for half in range(2):
                maps.append(_prep_core(x, prm, b, direction, half))
    return maps


def kernel(**inputs):
    inputs = {k: np.asarray(v, dtype=np.float32) for k, v in inputs.items()}
    nc = _CACHE.get("nc")
    if nc is None:
        nc = _build()
        _CACHE["nc"] = nc
    maps = _in_maps(inputs)
    res = run_bass_kernel_spmd(nc, maps, list(range(8)),
                               **_CACHE.get("run_kwargs", {}))
    _CACHE["last_results"] = res
    out = np.zeros((BATCH, SEQ, D_MODEL), dtype=np.float32)
    ci = 0
    for b in range(BATCH):
        for direction in range(2):
            for half in range(2):
                part = res.results[ci]["outp"].T          # (SEQ, D_MODEL)
                if direction == 1:
                    part = part[::-1]
                out[b] += part
                ci += 1
    return out
